# revision 30
# baseline (speedup 1.0000x reference)
"""AdaptiveRSNNEmbedding Trainium2 kernel v2 (8 NeuronCores, batch-parallel).

Reference semantics (per batch element, T time-reversed steps):
    g, c   = split(conv3x3(spike_prev, w_gate) + conv3x3(ev_t, w_in) + biases)
    gate   = sigmoid(g);  v = gate*vmem + c
    spike  = (v > 0.5);   vmem' = v - 0.5*spike
    agg[seg] accumulates vavg at spikes (seg<4), seg += spike, plus a final
    flush of the unclosed segment.

Identity: agg[s] += v_t * (seg_t == s) each step (seg_t = #spikes before t,
uncapped) replaces vavg/scatter/final-flush exactly.

v2 design vs v1:
  * fp16 hi+lo conv everywhere (1 cyc/row on PE, same as f32r):
      pass0 stationary = [wg_hi (96 rows) | w_in_hi (12 rows)]
      pass1 stationary = [wg_lo | w_in_lo]
    moving rows 0:96 = ky-stacked 0.5*spike (fp16 exact); rows 96:108 =
    events hi(2cin)+lo(2cin) x 3ky host-stacked planes. The separate f32
    event matmul of v1 (4 cyc/row!) disappears into the spare K rows.
  * ky-stacking at half-step granularity: per (t, half of 20 rows) 12 gather
    DMAs [32, 20*WP] (ky,hb) spk->ss + 1 event DMA; ~27 DMAs/step total vs
    ~126 (HWDGE descriptor-gen is a single shared device, ~0.65us per DMA).
  * agg planes bf16, SBUF-resident (v1 streamed agg[3] through DRAM);
    bf16+bf16 adds hit the DVE 4x mode. Host converts output to f32.
  * psum hb-paired: matmul hb0 -> partitions 0:64, hb1 -> 64:128 of one
    2-bank tile (tile_position=(0,64)); 2 tags x 2 bufs = 8 banks, full
    double buffering.
"""
import sys
import time
import numpy as np

if '/opt/trn_rl_repo' not in sys.path:
    sys.path.insert(0, '/opt/trn_rl_repo')

import concourse.bass as bass
import concourse.mybir as mybir
from concourse.tile import TileContext

F32 = mybir.dt.float32
F16 = mybir.dt.float16
BF16 = mybir.dt.bfloat16
U8 = mybir.dt.uint8
AF = mybir.ActivationFunctionType
OP = mybir.AluOpType

# problem constants
B, T, CIN, COUT, H, W = 16, 16, 2, 32, 160, 160
TSEG = 4
NCORES = 8
BL = B // NCORES
THRESH = 0.5

# kept for test.py compatibility
CONV_DT = F32
CONV_MODE = "fp16hl"


def mkap(base_ap, offset, dims):
    return type(base_ap)(base_ap.tensor, offset, [list(d) for d in dims])


def build_nc(BL=BL, T=T, H=H, W=W, conv_dt=None, mode=None):
    HB = 4
    BR = H // HB                # rows per block (40)
    HR = BR // 2                # rows per half-step (20)
    NPIX = BR * W               # per-partition state pixels (6400)
    WP = W + 2                  # padded row width (162)
    HP = H + 2                  # padded event-plane rows
    SR = max(1, 512 // W)       # matmul slice rows (psum bank limit)
    SSF = HR * WP               # ss cols per hb (3240)

    chunks = []                 # (c0, cr) within a half
    r = 0
    while r < HR:
        cr = min(2 * SR, HR - r)
        chunks.append((r, cr))
        r += cr

    nc = bass.Bass()
    nop_sem = nc.semaphore("nopsem").__enter__()
    ev_d = nc.declare_dram_parameter("ev", [BL * T, 12, HP * WP], F16,
                                     isOutput=False)
    ww_d = nc.declare_dram_parameter("ww", [128, 384], F16, isOutput=False)
    bgc_d = nc.declare_dram_parameter("bgc", [32, 2], F32, isOutput=False)
    out_d = nc.declare_dram_parameter("out", [TSEG, BL, COUT, H, W], BF16,
                                      isOutput=True)

    with TileContext(nc) as tc:
        with tc.tile_pool(name="const", bufs=1) as cpool, \
             tc.tile_pool(name="state", bufs=1) as spool, \
             tc.tile_pool(name="w1", bufs=3) as w1pool, \
             tc.tile_pool(name="wv", bufs=4) as vpool, \
             tc.tile_pool(name="wmv", bufs=1) as mvpool, \
             tc.tile_pool(name="psum", bufs=2, space="PSUM") as ppool:

            ww_t = cpool.tile([128, 384], F16, tag="ww")
            nc.sync.dma_start(out=ww_t[:], in_=ww_d[:])
            bgc_t = cpool.tile([32, 2], F32, tag="bgc")
            nc.sync.dma_start(out=bgc_t[:], in_=bgc_d[:])
            bg_t = bgc_t[:, 0:1]
            bc_t = bgc_t[:, 1:2]

            vmem = spool.tile([128, NPIX], F32, tag="vmem")
            seg = spool.tile([128, NPIX], F16, tag="seg")
            aggs = [spool.tile([128, NPIX], BF16, tag=f"agg{s}",
                               name=f"agg{s}") for s in range(TSEG)]
            spk = spool.tile([128, (BR + 2) * WP], F16, tag="spk")
            sss = [spool.tile([108, 4 * SSF], F16, tag=f"ss{hf}",
                              name=f"ss{hf}") for hf in range(2)]

            nc.nop_sem_num = nop_sem.num
            spk_v = spk[:].rearrange("p (r x) -> p r x", x=WP)

            # pad cols/rows of spk must read 0; interior rows are fully
            # rewritten at t==0 of every b, so one memset suffices globally
            nc.gpsimd.memset(spk[:].bitcast(F32), 0.0)

            for b in range(BL):

                def issue_gather(hf, ky, rlo, rhi):
                    # ss rows [rlo, rhi) of ky plane <- spk rows h0+ky+r
                    ss = sss[hf]
                    h0 = hf * HR
                    for hb in range(4):
                        nc.sync.dma_start(
                            out=ss[32 * ky:32 * (ky + 1),
                                   hb * SSF + rlo * WP:hb * SSF + rhi * WP],
                            in_=spk[32 * hb:32 * (hb + 1),
                                    (h0 + ky + rlo) * WP:
                                    (h0 + ky + rhi) * WP])

                def issue_top_edge():
                    # ssA ky0 row 0 <- block hb-1 row 39 (spk row 40), from
                    # the source partitions directly (no guard-row writes)
                    ss = sss[0]
                    for hb in range(4):
                        src = (spk[0:32, 0:WP] if hb == 0 else
                               spk[32 * (hb - 1):32 * hb,
                                   BR * WP:(BR + 1) * WP])
                        nc.sync.dma_start(
                            out=ss[0:32, hb * SSF:hb * SSF + WP], in_=src)

                def issue_bot_edge():
                    # ssB ky2 row HR-1 <- block hb+1 row 0 (spk row 1)
                    ss = sss[1]
                    for hb in range(4):
                        src = (spk[96:128, (BR + 1) * WP:(BR + 2) * WP]
                               if hb == 3 else
                               spk[32 * (hb + 1):32 * (hb + 2), WP:2 * WP])
                        nc.sync.dma_start(
                            out=ss[64:96,
                                   hb * SSF + (HR - 1) * WP:(hb + 1) * SSF],
                            in_=src)

                def issue_ev(hf, bt):
                    ss = sss[hf]
                    h0 = hf * HR
                    in_ev = mkap(
                        ev_d[:], bt * 12 * HP * WP + h0 * WP,
                        [(HP * WP, 12), (BR * WP, 4), (1, SSF)])
                    out_ev = ss[96:108, :].rearrange(
                        "p (hb rx) -> p hb rx", hb=4, rx=SSF)
                    nc.sync.dma_start(out=out_ev, in_=in_ev)

                def issue_out(h0w, h1w, last=False):
                    eng = nc.sync if last else nc.gpsimd
                    for s in range(TSEG):
                        for hb in range(4):
                            eng.dma_start(
                                out=out_d[s, b, :,
                                          hb * BR + h0w:hb * BR + h1w, :],
                                in_=aggs[s][32 * hb:32 * (hb + 1),
                                            h0w * W:h1w * W])

                def do_phase2(t, sl, F, v_t, spk_sl):
                    nc.gpsimd.tensor_tensor(vmem[:, sl], v_t[:, :F],
                                            spk_sl, OP.subtract)
                    if t == 0:
                        nc.vector.tensor_scalar(
                            aggs[0][:, sl], v_t[:, :F], 1.0, None,
                            OP.mult, OP.bypass)
                    else:
                        ns = min(t, TSEG - 1) + 1
                        for s in range(ns):
                            if s == t:
                                nc.vector.scalar_tensor_tensor(
                                    aggs[s][:, sl], seg[:, sl],
                                    0.5 * s, v_t[:, :F],
                                    OP.is_equal, OP.mult)
                            else:
                                mv_t = mvpool.tile([128, F], BF16,
                                                   tag=f"mv{s}")
                                nc.vector.scalar_tensor_tensor(
                                    mv_t[:, :F], seg[:, sl], 0.5 * s,
                                    v_t[:, :F], OP.is_equal, OP.mult)
                                nc.gpsimd.tensor_tensor(aggs[s][:, sl],
                                                        aggs[s][:, sl],
                                                        mv_t[:, :F], OP.add)
                    if t == 0:
                        nc.scalar.activation(seg[:, sl], spk_sl, AF.Copy)
                    else:
                        nc.gpsimd.tensor_tensor(seg[:, sl], seg[:, sl],
                                                spk_sl, OP.add)

                # p2(t, c) must issue before p1(t+1, c): lag strictly
                # less than chunks-per-step (and < vpool bufs)
                PKEEP = min(5, 2 * len(chunks) - 1)
                pending = []    # deferred phase-2 work

                def drain_pending(keep):
                    while len(pending) > keep:
                        args = pending.pop(0)
                        do_phase2(*args)

                for t in range(T):
                    bt = b * T + t
                    if t == 0:
                        issue_ev(0, bt)
                        issue_ev(1, bt)

                    for hf in range(2):
                        ss = sss[hf]
                        h0 = hf * HR
                        ss_r = ss[0:108, :].rearrange(
                            "p (hb r x) -> p hb r x", hb=4, r=HR, x=WP)
                        if len(chunks) > 2:
                            corder = ([chunks[1], chunks[0]]
                                      + chunks[2:])
                        elif len(chunks) == 2:
                            corder = [chunks[1], chunks[0]]
                        else:
                            corder = chunks
                        for ci, (c0, cr) in enumerate(corder):
                            F = cr * W
                            g0 = h0 + c0            # block-row of chunk start
                            sl = slice(g0 * W, g0 * W + F)
                            nsl = (cr + SR - 1) // SR

                            # ---- conv ----
                            # t==0: spikes are all zero; only the event rows
                            # (96:108) contribute -> K=12 matmuls
                            kb = 96 if t == 0 else 0
                            ps_ts = []
                            for hp_ in range(2):
                                ps = ppool.tile([128, nsl * 512], F32,
                                                tag=f"ps{hp_}")
                                ps_ts.append(ps)
                                for hbi in range(2):
                                    hb = 2 * hp_ + hbi
                                    tp = (kb, 64 * hbi)
                                    if kb == 0 and hbi == 0:
                                        tp = None
                                    pb = 64 * hbi
                                    nr, isl = 0, 0
                                    while nr < cr:
                                        srr = min(SR, cr - nr)
                                        out_ap = ps[pb:pb + 64,
                                                    isl * 512:
                                                    isl * 512 + srr * W]
                                        first = True
                                        for p_ in range(2):
                                            for kx in range(3):
                                                mv_ap = ss_r[kb:108, hb,
                                                             c0 + nr:
                                                             c0 + nr + srr,
                                                             kx:kx + W]
                                                nc.tensor.matmul(
                                                    out_ap,
                                                    ww_t[kb:108,
                                                         64 * (3 * p_ + kx):
                                                         64 * (3 * p_ + kx + 1)],
                                                    mv_ap,
                                                    start=first,
                                                    stop=(p_ == 1 and kx == 2),
                                                    tile_position=tp)
                                                first = False
                                        nr += srr
                                        isl += 1

                            # ---- extraction (phase 1) ----
                            # sigmoid skipped at t==0 (vmem==0, gate unused)
                            cur_t = w1pool.tile([128, F], F32, tag="cur")
                            gate_t = None
                            if t > 0:
                                gate_t = w1pool.tile([128, F], F32,
                                                     tag="gate")
                            full = (cr == nsl * SR)
                            for hb in range(4):
                                ps = ps_ts[hb // 2]
                                pb = 64 * (hb % 2)
                                gsl = slice(32 * hb, 32 * (hb + 1))
                                if full and nsl > 1:
                                    ps_g = ps[pb:pb + 32, :].rearrange(
                                        "p (n x) -> p n x",
                                        x=512)[:, 0:nsl, 0:SR * W]
                                    ps_c = ps[pb + 32:pb + 64, :].rearrange(
                                        "p (n x) -> p n x",
                                        x=512)[:, 0:nsl, 0:SR * W]
                                    c_o = cur_t[gsl, :].rearrange(
                                        "p (n x) -> p n x", x=SR * W)
                                    if t > 0:
                                        g_o = gate_t[gsl, :].rearrange(
                                            "p (n x) -> p n x", x=SR * W)
                                        nc.scalar.activation(g_o, ps_g,
                                                             AF.Sigmoid,
                                                             bias=bg_t)
                                    if hb == 0:
                                        nc.vector.tensor_scalar(
                                            c_o, ps_c, bc_t, None,
                                            OP.add, OP.bypass)
                                    else:
                                        nc.scalar.activation(c_o, ps_c,
                                                             AF.Identity,
                                                             bias=bc_t)
                                else:
                                    nr, isl = 0, 0
                                    while nr < cr:
                                        srr = min(SR, cr - nr)
                                        o0, o1 = nr * W, (nr + srr) * W
                                        p0 = isl * 512
                                        if t > 0:
                                            nc.scalar.activation(
                                                gate_t[gsl, o0:o1],
                                                ps[pb:pb + 32,
                                                   p0:p0 + srr * W],
                                                AF.Sigmoid, bias=bg_t)
                                        if hb == 0:
                                            nc.vector.tensor_scalar(
                                                cur_t[gsl, o0:o1],
                                                ps[pb + 32:pb + 64,
                                                   p0:p0 + srr * W],
                                                bc_t, None, OP.add, OP.bypass)
                                        else:
                                            nc.scalar.activation(
                                                cur_t[gsl, o0:o1],
                                                ps[pb + 32:pb + 64,
                                                   p0:p0 + srr * W],
                                                AF.Identity, bias=bc_t)
                                        nr += srr
                                        isl += 1

                            # ---- v + spike (phase 1, latency-critical:
                            # next step's gathers wait on the spikes) ----
                            if t == 0:
                                v_t = cur_t     # vmem==0 -> v = cur
                            else:
                                v_t = vpool.tile([128, F], F32, tag="v")
                                nc.vector.tensor_tensor(v_t[:, :F],
                                                        gate_t[:, :F],
                                                        vmem[:, sl], OP.mult)
                                nc.vector.tensor_tensor(v_t[:, :F],
                                                        v_t[:, :F],
                                                        cur_t[:, :F], OP.add)
                            spk_sl = spk_v[:, g0 + 1:g0 + 1 + cr, 1:1 + W]
                            nc.vector.tensor_scalar(spk_sl, v_t[:, :F],
                                                    THRESH, 0.5,
                                                    OP.is_gt, OP.mult)
                            # t==0: v_t aliases the bufs=2 cur tile, so its
                            # phase-2 use cannot be deferred across chunks
                            if t == 0:
                                do_phase2(t, sl, F, v_t, spk_sl)
                            else:
                                pending.append((t, sl, F, v_t, spk_sl))
                                drain_pending(PKEEP)



                        # ---- next-step gathers that only need this half
                        if t + 1 < T:
                            if hf == 0:
                                issue_gather(0, 1, 0, HR)
                                issue_gather(0, 0, 1, HR)
                                issue_gather(0, 2, 0, HR - 1)
                                issue_ev(0, bt + 1)
                            else:
                                issue_gather(0, 2, HR - 1, HR)
                                issue_top_edge()
                                issue_gather(1, 0, 0, HR)
                                issue_gather(1, 1, 0, HR)
                                issue_gather(1, 2, 0, HR - 1)
                                issue_bot_edge()
                                issue_ev(1, bt + 1)

                        if t == T - 1:
                            drain_pending(0)
                            issue_out(h0, h0 + HR,
                                      last=(b == BL - 1 and hf == 1))
    _split_matmul_waits(nc)
    return nc


def _split_matmul_waits(nc):
    """Walrus's LDW+MATMUL pair (and 2D DMA descriptors) have a single
    sync-wait slot; move extra waits onto same-engine no-ops inserted just
    before the instruction (safe: waits execute in order on the sequencer)."""
    nid = [0]
    for blk in nc.m.functions[0].blocks:
        out = []
        for inst in blk.instructions:
            si = inst.sync_info
            if (type(inst).__name__ != 'InstNoOp' and si is not None
                    and len(si.on_wait) > 1):
                keep = si.on_wait[-1:]
                for w in si.on_wait[:-1]:
                    nop = mybir.InstNoOp(name=f"NW-{nid[0]}", ins=[], outs=[])
                    nid[0] += 1
                    nop.engine = inst.engine
                    zupd = mybir.SyncUpdate(
                        sync_type='semaphore', id=nc.nop_sem_num,
                        ant_name='nopsem', update_mode='sem-inc',
                        update_value=1, update_reg=None)
                    nop.sync_info = mybir.SyncInfo(on_wait=[w],
                                                   on_update=[zupd])
                    out.append(nop)
                inst.sync_info = mybir.SyncInfo(on_wait=keep,
                                                on_update=si.on_update)
            out.append(inst)
        blk.instructions = out


def host_prep(events, w_in, b_in, w_gate, b_gate, conv_np=np.float32,
              ncores=NCORES, mode=None):
    """Build per-core input maps. events: [B,T,CIN,H,W] full."""
    Bf, Tf, Cf, Hf, Wf = events.shape
    HP, WP = Hf + 2, Wf + 2
    # time reversal + zero pad
    evr = events[:, ::-1].astype(np.float32)
    evp = np.zeros((Bf, Tf, Cf, HP, WP), np.float32)
    evp[..., 1:1 + Hf, 1:1 + Wf] = evr
    hi = evp.astype(np.float16)
    lo = (evp - hi.astype(np.float32)).astype(np.float16)
    # ky-shifted stacked planes: evs[:,:,3*ch+ky, r] = src_ch[r+ky]
    evs = np.zeros((Bf, Tf, 12, HP, WP), np.float16)
    for ch in range(4):
        src = hi[:, :, ch] if ch < 2 else lo[:, :, ch - 2]
        for ky in range(3):
            evs[:, :, 3 * ch + ky, 0:HP - ky] = src[:, :, ky:HP]

    # stationary weights [128, 384]: col block bk = 3*pass + kx
    w2 = 2.0 * np.asarray(w_gate, np.float32)          # [64, 32, 3, 3]
    w2hi = w2.astype(np.float16)
    w2lo = (w2 - w2hi.astype(np.float32)).astype(np.float16)
    wi = np.asarray(w_in, np.float32)                  # [64, 2, 3, 3]
    wih = wi.astype(np.float16)
    wil = (wi - wih.astype(np.float32)).astype(np.float16)
    ww = np.zeros((128, 384), np.float16)
    for p_ in range(2):
        wg_src = w2hi if p_ == 0 else w2lo
        wi_src = wih if p_ == 0 else wil
        for kx in range(3):
            c0 = 64 * (3 * p_ + kx)
            for ky in range(3):
                for c in range(COUT):
                    ww[32 * ky + c, c0:c0 + 64] = wg_src[:, c, ky, kx]
                for ch in range(4):
                    cin = ch % 2
                    ww[96 + 3 * ch + ky, c0:c0 + 64] = wi_src[:, cin, ky, kx]
    bgc = np.stack([np.asarray(b_gate[:32] + b_in[:32]),
                    np.asarray(b_gate[32:] + b_in[32:])],
                   axis=1).astype(np.float32)

    bl = Bf // ncores
    in_maps = []
    for i in range(ncores):
        ev_i = evs[i * bl:(i + 1) * bl].reshape(bl * Tf, 12, HP * WP)
        in_maps.append({"ev": np.ascontiguousarray(ev_i), "ww": ww,
                        "bgc": bgc})
    return in_maps


_cache = {}
last_run_info = {}


def kernel(events, w_in, b_in, w_gate, b_gate, trace=False):
    from concourse import bass_utils
    key = ("v2",)
    if key not in _cache:
        _cache[key] = build_nc()
    nc = _cache[key]
    in_maps = host_prep(np.asarray(events), np.asarray(w_in),
                        np.asarray(b_in), np.asarray(w_gate),
                        np.asarray(b_gate))
    t0 = time.time()
    res = bass_utils.run_bass_kernel_spmd(
        nc, in_maps, core_ids=list(range(NCORES)), trace=trace)
    wall = time.time() - t0
    last_run_info.update(exec_time_ns=res.exec_time_ns, wall_s=wall,
                         profile_json=getattr(res, "profile_json", None))
    outs = [np.asarray(res.results[i]["out"]).astype(np.float32)
            for i in range(NCORES)]
    return np.concatenate(outs, axis=1)
